# revision 1
# baseline (speedup 1.0000x reference)
"""Trainium2 Bass kernel for EquiMAB3-style attention block (v2).

Reference computation (per batch b; B=8, N=M=512, S=4, D=L=256, H=4, dh=64):
  Qp = Q @ Wq.T + bq          [N,S,L]
  Kp = K @ Wk.T + bk          [M,S,L]
  Vp = K @ Wv.T + bv          [M,S,L]
  E[h,n,m] = sum_{s,j} Qp[n,s,h*64+j] Kp[m,s,h*64+j] / 16
  A = softmax_m(E)  (mask all ones in setup -> no-op; fallback handled)
  O[n,s,l=h*64+j] = Qp[n,s,l] + sum_m A[h,n,m] Vp[m,s,l]
  O = LN0(O)*g0+b0 ; O = O + relu(O @ Wo.T + bo) ; out = LN1(O)*g1+b1

Sharding: data-parallel over B across the 8 cores (one batch element each).

v2 design notes (cost-model-driven):
  - token index is s-major on device: t = s*512 + n  (host pre-transposes)
  - matmul cost = out_free_rows * pe_cycle; PE p-state needs continuous busy
    => no big warmup bursts; dense dependency-fed PE stream; tiny bridges
  - E^T psums hold a head-pair [m, (hh, n)] so exp evacs 1024 rows at a time
  - softmax denominator via ones-column embedded in VA (m-major Vp)
  - bv folded into VA psum via K=1 ones-row matmuls (softmax rows sum to 1)
  - fused evac: o = (avn_psum * recip) + qn_psum via scalar_tensor_tensor
  - fc computed with n-major output tiles [t, l'] so LN1 needs no
    transpose-back; relu+residual fused in one STT; bo via K=1 matmul
  - 16 independent (s, nch) LN1 chains pipeline across DVE/ACT
"""

import os
import numpy as np

B, N, M, S, D, L, H = 8, 512, 512, 4, 256, 256, 4
DH = L // H  # 64
T = N * S  # 2048 tokens per core
P = 128
NCORES = 8

_CACHE = {}


def _build(use_mask, aff0_trivial, aff1_trivial):
    from contextlib import ExitStack

    import concourse.bacc as bacc
    import concourse.bass as bass
    import concourse.mybir as mybir
    import concourse.tile as tile

    f32 = mybir.dt.float32
    b16 = mybir.dt.bfloat16
    AF = mybir.ActivationFunctionType
    OP = mybir.AluOpType

    nc = bacc.Bacc(
        "TRN2",
        target_bir_lowering=False,
        debug=False,
        enable_asserts=False,
        num_devices=NCORES,
    )

    def dram(name, shape, kind="ExternalInput", dt=None):
        return nc.dram_tensor(name, shape, dt or f32, kind=kind).ap()

    qt_d = dram("qt", [D, T], dt=b16)        # [d, s*512+n]
    kt_d = dram("kt", [D, T], dt=b16)        # [d, s*512+m]
    wall_d = dram("wall", [4, D, L], dt=b16)  # WqT, WkT, WvT, WoT stacked
    bqkc_d = dram("bqkc", [P, 2, 2])          # bias cols: [p, lc, (bq|bk)] f32
    onr_d = dram("onr", [1, P], dt=b16)       # ones row (K=1 matmul lhsT)
    bv2_d = dram("bv2", [1, 2 * L], dt=b16)   # [bv, bv] row
    bor_d = dram("bor", [1, L], dt=b16)       # bo row
    identr_d = dram("identr", [P, P], dt=b16)
    if use_mask:
        mbt_d = dram("mbt", [M, N])           # (mask==0 ? -1e30 : 0).T  [m, n]
    if not aff0_trivial:
        g0r_d = dram("g0r", [L], dt=b16)
        b0r_d = dram("b0r", [L], dt=b16)
    if not aff1_trivial:
        g1_d = dram("g1v", [L])
        b1_d = dram("b1v", [L])
    out_d = dram("o", [N, S * L], kind="ExternalOutput")

    with tile.TileContext(nc) as tc, ExitStack() as ctx:
        singles = ctx.enter_context(tc.tile_pool(name="singles", bufs=1))
        iop = ctx.enter_context(tc.tile_pool(name="iop", bufs=4))
        fmaj = ctx.enter_context(tc.tile_pool(name="fmaj", bufs=4))
        nmaj = ctx.enter_context(tc.tile_pool(name="nmaj", bufs=4))
        atp = ctx.enter_context(tc.tile_pool(name="atp", bufs=8))
        vap = ctx.enter_context(tc.tile_pool(name="vap", bufs=4))
        stat = ctx.enter_context(tc.tile_pool(name="stat", bufs=1))
        # PSUM banks: ppA = [128,1024] f32 (2 banks) x2 = 4; ppq = [128,1024]
        # b16 (1 bank) x2 = 2; ppv = [128,260] f32 x2 = 2  => 8 banks total
        ppA = ctx.enter_context(tc.tile_pool(name="ppA", bufs=2, space="PSUM"))
        ppq = ctx.enter_context(tc.tile_pool(name="ppq", bufs=2, space="PSUM"))
        ppv = ctx.enter_context(tc.tile_pool(name="ppv", bufs=2, space="PSUM"))

        def mm(out, lhsT, rhs, **kw):
            nc.tensor.matmul(out, lhsT, rhs, **kw)

        # ---- small constant DMAs (gpsimd queues: cheap dispatch) ----
        ident_r = singles.tile([P, P], b16)
        nc.scalar.dma_start(out=ident_r, in_=identr_d)
        onr_sb = singles.tile([1, P], b16)
        nc.scalar.dma_start(out=onr_sb, in_=onr_d)
        bv2_sb = singles.tile([1, 2 * L], b16)
        nc.scalar.dma_start(out=bv2_sb, in_=bv2_d)
        bor_sb = singles.tile([1, L], b16)
        nc.gpsimd.dma_start(out=bor_sb, in_=bor_d)
        bqkc_sb = singles.tile([P, 2, 2], f32)
        nc.scalar.dma_start(out=bqkc_sb, in_=bqkc_d)
        eps_col = singles.tile([P, 1], f32)
        nc.vector.memset(eps_col, 1e-5)
        warm_sb = singles.tile([P, 512], b16)
        nc.vector.memset(warm_sb, 0.0)

        if not aff0_trivial:
            g0_bc = singles.tile([P, S, L], b16)
            b0_bc = singles.tile([P, S, L], b16)
            g0_rep = bass.AP(tensor=g0r_d.tensor, offset=g0r_d.offset,
                             ap=[[0, P], [0, S]] + [list(d) for d in g0r_d.ap])
            b0_rep = bass.AP(tensor=b0r_d.tensor, offset=b0r_d.offset,
                             ap=[[0, P], [0, S]] + [list(d) for d in b0r_d.ap])
            nc.gpsimd.dma_start(out=g0_bc, in_=g0_rep)
            nc.gpsimd.dma_start(out=b0_bc, in_=b0_rep)
        if not aff1_trivial:
            g1_bc = singles.tile([P, L], f32)
            b1_bc = singles.tile([P, L], f32)
            g1_rep = bass.AP(tensor=g1_d.tensor, offset=g1_d.offset,
                             ap=[[0, P]] + [list(d) for d in g1_d.ap])
            b1_rep = bass.AP(tensor=b1_d.tensor, offset=b1_d.offset,
                             ap=[[0, P]] + [list(d) for d in b1_d.ap])
            nc.gpsimd.dma_start(out=g1_bc, in_=g1_rep)
            nc.gpsimd.dma_start(out=b1_bc, in_=b1_rep)

        # ---- bulk input DMAs (sync/SP queue, HWDGE): weights FIRST ----
        wall_sb = singles.tile([P, 2, 4, L], b16)
        wall_r = wall_d.rearrange("w (c p) l -> p c w l", p=P)
        for c in range(2):
            nc.sync.dma_start(out=wall_sb[:, c, :, :], in_=wall_r[:, c, :, :])

        qt_sb = [iop.tile([P, T], b16, tag="io", name=f"qt{i}") for i in range(2)]
        kt_sb = [iop.tile([P, T], b16, tag="io", name=f"kt{i}") for i in range(2)]
        for half in range(2):
            t0, t1 = half * 1024, (half + 1) * 1024
            for dc in range(2):
                nc.sync.dma_start(out=qt_sb[dc][:, t0:t1],
                                  in_=qt_d[dc * P:(dc + 1) * P, t0:t1])
            for dc in range(2):
                nc.sync.dma_start(out=kt_sb[dc][:, t0:t1],
                                  in_=kt_d[dc * P:(dc + 1) * P, t0:t1])

        if use_mask:
            mb_sb = [singles.tile([P, N], f32, tag="mb", name=f"mb{i}")
                     for i in range(4)]
            for mc in range(4):
                nc.sync.dma_start(out=mb_sb[mc], in_=mbt_d[mc * P:(mc + 1) * P, :])

        wq_sb = wall_sb[:, :, 0, :]
        wk_sb = wall_sb[:, :, 1, :]
        wv_sb = wall_sb[:, :, 2, :]
        wo_sb = wall_sb[:, :, 3, :]

        # ---- PE ramp warmers (512-row matmuls into the ppq pool, b16) ----
        def warm(n):
            for _ in range(n):
                pw = ppv.tile([P, S, DH + 1], f32, tag="pavn", name="pw")
                mm(pw, warm_sb[:, 0:P], warm_sb[:, 0:260],
                   start=True, stop=True)

        warm(16)

        # ---- projections QpT / KpT: [l, t] = W^T.T @ X^T, s-major t ----
        qpt = [fmaj.tile([P, T], b16, tag="fm", name=f"qpt{i}") for i in range(2)]
        kpt = [fmaj.tile([P, T], b16, tag="fm", name=f"kpt{i}") for i in range(2)]

        for half in range(2):  # t-halves (s-pairs)
            t0 = half * 1024
            for lc in range(2):
                # Q -> ACT evac (+bq)
                pq = ppA.tile([P, 1024], f32, tag="pA", name="pq")
                for k in range(2):  # s within half
                    for dc in range(2):
                        mm(pq[:, k * 512:(k + 1) * 512],
                           wq_sb[:, dc, lc * P:(lc + 1) * P],
                           qt_sb[dc][:, t0 + k * 512: t0 + (k + 1) * 512],
                           start=(dc == 0), stop=(dc == 1))
                nc.scalar.activation(qpt[lc][:, t0:t0 + 1024], pq, AF.Identity,
                                     bias=bqkc_sb[:, lc, 0:1])
                # K -> DVE evac (+bk)
                pk = ppA.tile([P, 1024], f32, tag="pA", name="pk")
                for k in range(2):
                    for dc in range(2):
                        mm(pk[:, k * 512:(k + 1) * 512],
                           wk_sb[:, dc, lc * P:(lc + 1) * P],
                           kt_sb[dc][:, t0 + k * 512: t0 + (k + 1) * 512],
                           start=(dc == 0), stop=(dc == 1))
                nc.vector.tensor_scalar(
                    out=kpt[lc][:, t0:t0 + 1024], in0=pk,
                    scalar1=bqkc_sb[:, lc, 1:2], scalar2=None, op0=OP.add)

        # ---- VA: m-major Vp (+bv via K=1 ones matmuls), ones col for denom ----
        va = [vap.tile([P, S, H, DH + 1], b16, tag="va", name=f"va{i}")
              for i in range(4)]
        for mch in range(4):
            nc.gpsimd.memset(va[mch][:, :, :, DH:DH + 1], 1.0)
            pv = ppA.tile([P, S, 256], f32, tag="pA", name="pv")
            for s in range(S):
                # bv seed: K=1 ones-row matmul opens this s-group
                mm(pv[:, s, :], onr_sb, bv2_sb[:, 0:L], start=True, stop=False)
                for dc in range(2):
                    mm(pv[:, s, :],
                       kt_sb[dc][:, s * 512 + mch * P: s * 512 + (mch + 1) * P],
                       wv_sb[:, dc, :], start=False, stop=(dc == 1))
            nc.vector.tensor_copy(
                va[mch][:, :, :, 0:DH],
                pv.rearrange("p s (h j) -> p s h j", j=DH))

        # ---- E^T per head-pair: psum [m, (hh, n)]; exp -> at tiles ----
        at = {}
        for hp in range(2):
            lc = hp
            for mc in range(4):
                pe = ppA.tile([P, 2, N], f32, tag="pA", name="pe")
                for hh in range(2):
                    r0 = hh * DH
                    for s in range(S):
                        mm(pe[:, hh, :],
                           kpt[lc][r0:r0 + DH,
                                   s * 512 + mc * P: s * 512 + (mc + 1) * P],
                           qpt[lc][r0:r0 + DH, s * 512: (s + 1) * 512],
                           start=(s == 0), stop=(s == S - 1))
                if use_mask:
                    for hh in range(2):
                        nc.vector.tensor_tensor(
                            out=pe[:, hh, :], in0=pe[:, hh, :], in1=mb_sb[mc],
                            op=OP.add)
                a = atp.tile([P, 2, N], b16, tag="at", name=f"at{hp}{mc}")
                nc.scalar.activation(a, pe, AF.Exp, scale=1.0 / 16.0)
                at[(hp, mc)] = a

        # ---- software-pipelined per-nch: attention(k) || tail(k-1) ----
        o_sb = [nmaj.tile([P, S, L], b16, tag="nm", name=f"o{i}") for i in range(4)]
        mv0 = stat.tile([P, 16, 2], f32)
        tmp0 = stat.tile([P, 16], f32)
        rstd0 = stat.tile([P, 16], f32)
        nm0 = stat.tile([P, 16], f32)
        ybf = [nmaj.tile([P, S, L], b16, tag="nm2", name=f"ybf{i}")
               for i in range(4)]
        o1n = ybf if aff0_trivial else [
            nmaj.tile([P, S, L], b16, tag="nm4", name=f"o1n{i}") for i in range(4)]
        o1t = [fmaj.tile([P, T], b16, tag="fm2", name=f"o1t{i}") for i in range(2)]
        mv1 = stat.tile([P, 16, 2], f32)
        sig1 = stat.tile([P, 16], f32)
        rstd1 = stat.tile([P, 16], f32)
        nm1 = stat.tile([P, 16], f32)
        o2 = [nmaj.tile([P, S, L], b16, tag="nm3", name=f"o2_{i}")
              for i in range(4)]
        out_sb = [nmaj.tile([P, S, L], f32, tag="nm5", name=f"os{i}")
                  for i in range(4)]

        def attention_part(nch):
            # QN: transpose Qp slices into a b16 psum [n, (s,l)]
            qn = ppq.tile([P, 1024], b16, tag="qn", name="qn")
            for s in range(S):
                for lc in range(2):
                    mm(qn[:, s * 256 + lc * P: s * 256 + (lc + 1) * P],
                       qpt[lc][:, s * 512 + nch * P: s * 512 + (nch + 1) * P],
                       ident_r, is_transpose=True, start=True, stop=True)
            # seed o with the transposed Qp (ACT copy b16 psum -> SBUF)
            nc.scalar.copy(o_sb[nch].rearrange("p s l -> p (s l)"), qn)
            # AVn per head + fused (AV/denom + o_seed) evac, in-place
            for h in range(H):
                pa = ppv.tile([P, S, DH + 1], f32, tag="pavn", name="pa")
                for mc in range(4):
                    mm(pa, at[(h // 2, mc)][:, h % 2, nch * P:(nch + 1) * P],
                       va[mc][:, :, h, :], start=(mc == 0), stop=(mc == 3))
                rc = stat.tile([P, 1], f32, tag="rc", bufs=4, name="rc")
                nc.vector.reciprocal(rc, pa[:, 0, DH:DH + 1])
                nc.vector.scalar_tensor_tensor(
                    out=o_sb[nch][:, :, h * DH:(h + 1) * DH],
                    in0=pa[:, :, 0:DH], scalar=rc,
                    in1=o_sb[nch][:, :, h * DH:(h + 1) * DH],
                    op0=OP.mult, op1=OP.add)
            # LN0 stats (DVE) + normalize (GpSimd)
            for s in range(S):
                st6 = stat.tile([P, 6], f32, tag="st6", bufs=4)
                nc.vector.bn_stats(out=st6, in_=o_sb[nch][:, s, :])
                nc.vector.bn_aggr(out=mv0[:, nch * 4 + s, :], in_=st6)
            sl0 = slice(nch * 4, nch * 4 + 4)
            nc.scalar.activation(tmp0[:, sl0], mv0[:, sl0, 1],
                                 AF.Sqrt, bias=eps_col)
            nc.vector.reciprocal(rstd0[:, sl0], tmp0[:, sl0])
            nc.vector.scalar_tensor_tensor(
                out=nm0[:, sl0], in0=mv0[:, sl0, 0], scalar=-1.0,
                in1=rstd0[:, sl0], op0=OP.mult, op1=OP.mult)
            for s in range(S):
                i = nch * 4 + s
                if s < 2:
                    nc.vector.tensor_scalar(
                        out=ybf[nch][:, s, :], in0=o_sb[nch][:, s, :],
                        scalar1=mv0[:, i, 0:1], scalar2=rstd0[:, i:i + 1],
                        op0=OP.subtract, op1=OP.mult)
                else:
                    nc.scalar.activation(
                        ybf[nch][:, s, :], o_sb[nch][:, s, :], AF.Identity,
                        bias=nm0[:, i:i + 1], scale=rstd0[:, i:i + 1])
            if not aff0_trivial:
                nc.gpsimd.scalar_tensor_tensor(
                    out=o1n[nch][:, :, :].rearrange("p s l -> p (s l)"),
                    in0=ybf[nch][:, :, :].rearrange("p s l -> p (s l)"),
                    scalar=1.0,
                    in1=g0_bc.rearrange("p s l -> p (s l)"),
                    op0=OP.bypass, op1=OP.mult)
                nc.vector.tensor_tensor(
                    out=o1n[nch][:, :, :].rearrange("p s l -> p (s l)"),
                    in0=o1n[nch][:, :, :].rearrange("p s l -> p (s l)"),
                    in1=b0_bc.rearrange("p s l -> p (s l)"), op=OP.add)

        def tail_part(nch):
            # O1T: transpose o1n[nch] -> per-nch psum [l, (lc, s, n-slice)]
            pt = ppq.tile([P, 8, P], b16, tag="qn", name="pt")
            for lc in range(2):
                for s in range(S):
                    mm(pt[:, lc * 4 + s, :],
                       o1n[nch][:, s, lc * P:(lc + 1) * P],
                       ident_r, is_transpose=True, start=True, stop=True)
            for lc in range(2):
                dst = o1t[lc].rearrange("p (s n) -> p s n", n=512)[
                    :, :, nch * P:(nch + 1) * P]
                if lc == 0:
                    nc.scalar.activation(dst, pt[:, 0:4, :], AF.Identity)
                else:
                    nc.vector.tensor_copy(dst, pt[:, 4:8, :])

            # fc (n-major out) + fused relu/residual + LN1 for this nch
            pf = ppA.tile([P, 4, 256], f32, tag="pA", name="pf")
            for s in range(4):
                tch = s * 4 + nch
                # bo seed opens the group; fc matmuls accumulate
                mm(pf[:, s, :], onr_sb, bor_sb, start=True, stop=False)
                for lc in range(2):
                    mm(pf[:, s, :],
                       o1t[lc][:, tch * P:(tch + 1) * P],
                       wo_sb[:, lc, :], start=False, stop=(lc == 1))
            i0 = nch * 4
            for s in range(4):
                if s % 2 == 0:
                    nc.vector.scalar_tensor_tensor(
                        out=o2[nch][:, s, :], in0=pf[:, s, :], scalar=0.0,
                        in1=o1n[nch][:, s, :], op0=OP.max, op1=OP.add)
                else:
                    rt = nmaj.tile([P, L], b16, tag="rt", bufs=2, name="rt")
                    nc.scalar.activation(rt, pf[:, s, :], AF.Relu)
                    nc.gpsimd.tensor_tensor(out=o2[nch][:, s, :], in0=rt,
                                            in1=o1n[nch][:, s, :], op=OP.add)
                st6 = stat.tile([P, 6], f32, tag="st6", bufs=4)
                nc.vector.bn_stats(out=st6, in_=o2[nch][:, s, :])
                nc.vector.bn_aggr(out=mv1[:, i0 + s, :], in_=st6)
            sl = slice(i0, i0 + 4)
            nc.scalar.activation(sig1[:, sl], mv1[:, sl, 1], AF.Sqrt,
                                 bias=eps_col)
            nc.vector.reciprocal(rstd1[:, sl], sig1[:, sl])
            nc.vector.scalar_tensor_tensor(
                out=nm1[:, sl], in0=mv1[:, sl, 0], scalar=-1.0,
                in1=rstd1[:, sl], op0=OP.mult, op1=OP.mult)
            for s in range(4):
                i = i0 + s
                if s % 2 == 0:
                    nc.vector.tensor_scalar(
                        out=out_sb[nch][:, s, :], in0=o2[nch][:, s, :],
                        scalar1=mv1[:, i, 0:1], scalar2=rstd1[:, i:i + 1],
                        op0=OP.subtract, op1=OP.mult)
                else:
                    nc.scalar.activation(
                        out_sb[nch][:, s, :], o2[nch][:, s, :], AF.Identity,
                        bias=nm1[:, i:i + 1], scale=rstd1[:, i:i + 1])
            if not aff1_trivial:
                for s in range(S):
                    if s % 2 == 0:
                        nc.vector.tensor_mul(out_sb[nch][:, s, :],
                                             out_sb[nch][:, s, :], g1_bc)
                        nc.gpsimd.tensor_add(out_sb[nch][:, s, :],
                                             out_sb[nch][:, s, :], b1_bc)
                    else:
                        nc.gpsimd.tensor_mul(out_sb[nch][:, s, :],
                                             out_sb[nch][:, s, :], g1_bc)
                        nc.vector.tensor_add(out_sb[nch][:, s, :],
                                             out_sb[nch][:, s, :], b1_bc)
            for s in range(S):
                nc.gpsimd.dma_start(
                    out=out_d[nch * P:(nch + 1) * P, s * L:(s + 1) * L],
                    in_=out_sb[nch][:, s, :])

        for k in range(5):
            if k < 4:
                attention_part(k)
            if k >= 1:
                warm(1)
                tail_part(k - 1)

    nc.compile()
    return nc


def kernel(**inputs):
    global _CACHE
    Q = np.asarray(inputs["Q"], dtype=np.float32)
    K = np.asarray(inputs["K"], dtype=np.float32)
    mask = np.asarray(inputs["mask"])
    Wq = np.asarray(inputs["Wq"], dtype=np.float32)
    bq = np.asarray(inputs["bq"], dtype=np.float32)
    Wk = np.asarray(inputs["Wk"], dtype=np.float32)
    bk = np.asarray(inputs["bk"], dtype=np.float32)
    Wv = np.asarray(inputs["Wv"], dtype=np.float32)
    bv = np.asarray(inputs["bv"], dtype=np.float32)
    Wo = np.asarray(inputs["Wo"], dtype=np.float32)
    bo = np.asarray(inputs["bo"], dtype=np.float32)
    g0 = np.asarray(inputs["g0"], dtype=np.float32)
    b0 = np.asarray(inputs["b0"], dtype=np.float32)
    g1 = np.asarray(inputs["g1"], dtype=np.float32)
    b1 = np.asarray(inputs["b1"], dtype=np.float32)

    use_mask = not bool((mask != 0).all())
    aff0_trivial = bool((g0 == 1.0).all() and (b0 == 0.0).all())
    aff1_trivial = bool((g1 == 1.0).all() and (b1 == 0.0).all())

    from concourse.bass_utils import run_bass_kernel_spmd

    key = ("nc", use_mask, aff0_trivial, aff1_trivial)
    if key not in _CACHE:
        _CACHE[key] = _build(use_mask, aff0_trivial, aff1_trivial)
    nc = _CACHE[key]

    import ml_dtypes
    bf16 = ml_dtypes.bfloat16
    common = {
        "identr": np.eye(P, dtype=bf16),
        "wall": np.stack([Wq.T, Wk.T, Wv.T, Wo.T]).astype(bf16),
        "bqkc": np.stack([bq.reshape(2, P).T, bk.reshape(2, P).T], axis=2),
        "onr": np.ones((1, P), dtype=bf16),
        "bv2": np.concatenate([bv, bv])[None, :].astype(bf16),
        "bor": bo[None, :].astype(bf16),
    }
    if use_mask:
        common["mbt"] = np.ascontiguousarray(
            np.where(mask == 0, np.float32(-1e30), np.float32(0.0)).T)
    if not aff0_trivial:
        common["g0r"] = g0.astype(bf16)
        common["b0r"] = b0.astype(bf16)
    if not aff1_trivial:
        common["g1v"] = g1
        common["b1v"] = b1

    in_maps = []
    for b in range(NCORES):
        m = dict(common)
        # s-major tokens: qt[d, s*512+n] = Q[b, n, s, d]
        m["qt"] = np.ascontiguousarray(
            Q[b].transpose(2, 1, 0).reshape(D, T)).astype(bf16)
        m["kt"] = np.ascontiguousarray(
            K[b].transpose(2, 1, 0).reshape(D, T)).astype(bf16)
        in_maps.append(m)

    trace = os.environ.get("KERNEL_TRACE", "0") == "1"
    res = run_bass_kernel_spmd(nc, in_maps, core_ids=list(range(NCORES)),
                               trace=trace)
    globals()["LAST_RESULTS"] = res
    out = np.stack([res.results[b]["o"].reshape(N, S, L) for b in range(NCORES)])
    return out

